# revision 2
# baseline (speedup 1.0000x reference)
"""Causal multi-head attention (B=32,T=512,C=1024,H=16,D=64) on 8 TRN2 cores.

v2: data-parallel over batch (4 per core). Differences vs v1:
  - AV computed transposed: outT[d,t] = sum_s V[s,d] attnT[s,t] with V
    stationary and attnT streaming (N=512..128) -> stream-dense PE, no
    LDWEIGHTS bottleneck, and the head-concat [hd, t] layout falls out
    directly (no DMA transposes).
  - V stationary is [ones*64 | V]: psum rows 0:64 get the softmax
    row-sums replicated by the PE for free; normalization is one fast
    approximate DVE reciprocal + a DVE/GPSIMD-split multiply per head.
  - scores+exp for head pair m-1 interleave into the QK projection
    phase, so ACT's exp work (~17us/batch) hides under the QK matmuls
    and the AV phase has no ACT dependency -> PE stays HAM-warm.
  - scores psum as 4 per-block tiles (bufs=1) = 4 banks; AV/proj psum
    share 2 banks; QKV chains 2 banks: exactly 8 PSUM banks.
"""

import sys

if "/opt/trn_rl_repo" not in sys.path:
    sys.path.insert(0, "/opt/trn_rl_repo")

import numpy as np
import ml_dtypes

B, T, C = 32, 512, 1024
H, D = 16, 64
HD = H * D
NCORES = 8
B_LOC = B // NCORES

_CACHE = {}


def build_nc(b_loc=B_LOC):
    import concourse.mybir as mybir
    from concourse import bacc
    from concourse.bass import ds, ts
    from concourse.tile import TileContext

    f32 = mybir.dt.float32
    bf16 = mybir.dt.bfloat16
    f8e4 = mybir.dt.float8e4
    DR = mybir.MatmulPerfMode.DoubleRow
    AF = mybir.ActivationFunctionType

    KO = C // 128  # 8 contraction chunks
    GO = C // 256  # 4 double-row contraction groups
    MO = HD // 128  # 8 output-row chunks
    TCH = T // 128  # 4 t-chunks
    SCALE = 1.0 / float(np.sqrt(C))
    SCALE_QK = SCALE / 1024.0

    widths = [T - 128 * j for j in range(TCH)]  # 512, 384, 256, 128
    # scores packed j0|j3|j1|j2 in ONE [128,1280] psum tile: every block's
    # matmul dst stays inside a 2KB bank (j0@[0,512) b0, j3@[512,640) b1,
    # j1@[640,1024) b1, j2@[1024,1280) b2) -> one exp per head.
    off = [0, 640, 1024, 512]

    nc = bacc.Bacc("TRN2", target_bir_lowering=False)
    xT = nc.dram_tensor("xT", [b_loc, C, T], bf16, kind="ExternalInput")
    x8 = nc.dram_tensor("x8", [b_loc, 128, GO, 2, T], f8e4, kind="ExternalInput")
    wq = nc.dram_tensor("wq", [128, GO, 2, HD], f8e4, kind="ExternalInput")
    wk = nc.dram_tensor("wk", [128, GO, 2, HD], f8e4, kind="ExternalInput")
    wv = nc.dram_tensor("wv", [C, HD], bf16, kind="ExternalInput")
    wp = nc.dram_tensor("wp", [C, C], bf16, kind="ExternalInput")
    bp = nc.dram_tensor("bp", [1, C], bf16, kind="ExternalInput")
    mask = nc.dram_tensor("mask", [128, 128], bf16, kind="ExternalInput")
    out = nc.dram_tensor("out", [b_loc, T, C], f32, kind="ExternalOutput")

    with TileContext(nc) as tc:
        with (
            tc.tile_pool(name="weights", bufs=1) as wpool,
            tc.tile_pool(name="acts", bufs=2) as xpool,
            tc.tile_pool(name="attn", bufs=1) as apool,
            tc.tile_pool(name="small", bufs=3) as spool,
            tc.tile_pool(name="outs", bufs=2) as opool,
            tc.tile_pool(name="psS", bufs=1, space="PSUM") as psS,
            tc.tile_pool(name="psAV", bufs=3, space="PSUM") as psA,
            tc.tile_pool(name="ps1", bufs=2, space="PSUM") as psB,
        ):
            # ---- persistent weights / constants ----
            wq_sb = wpool.tile([128, GO, 2, HD], f8e4, name="wq_sb")
            wk_sb = wpool.tile([128, GO, 2, HD], f8e4, name="wk_sb")
            wv_sb = wpool.tile([128, KO, HD], bf16, name="wv_sb")
            wp_sb = wpool.tile([128, KO, C], bf16, name="wp_sb")
            xT0_sb = xpool.tile([128, KO, T], bf16, name="xT0_sb", tag="xT")
            x80_sb = xpool.tile([128, GO, 2, T], f8e4, name="x80_sb", tag="x8")
            bp1_sb = wpool.tile([1, C], bf16, name="bp1_sb")
            nc.sync.dma_start(out=bp1_sb, in_=bp[:])
            mask_sb = wpool.tile([128, 128], bf16, name="mask_sb")
            nc.sync.dma_start(out=mask_sb, in_=mask[:])
            # fp8 x + wq first so the first Q chain starts ~4us in
            for g in range(GO):
                nc.sync.dma_start(out=x80_sb[:, g, :, :], in_=x8[0, :, g, :, :])
                nc.sync.dma_start(out=wq_sb[:, g, :, :], in_=wq[:, g, :, :])
            for g in range(GO):
                nc.sync.dma_start(out=wk_sb[:, g, :, :], in_=wk[:, g, :, :])
            for k in range(KO):
                nc.sync.dma_start(out=xT0_sb[:, k, :], in_=xT[0, ds(128 * k, 128), :])
            for k in range(KO):
                nc.sync.dma_start(out=wv_sb[:, k, :], in_=wv[ds(128 * k, 128), :])
            nc.sync.dma_start(
                out=wp_sb, in_=wp[:].rearrange("(ko p) n -> p ko n", p=128)
            )
            # bias broadcast to all 128 partitions, f32, built once
            ones1_sb = wpool.tile([1, 128], bf16, name="ones1_sb")
            nc.gpsimd.memset(ones1_sb, 1.0)
            bias_bc = wpool.tile([128, C], f32, name="bias_bc")
            for half in range(2):
                psb = psB.tile([128, 512], f32, name="psb", tag="ps1")
                nc.tensor.matmul(
                    psb, ones1_sb, bp1_sb[:, ts(half, 512)], start=True, stop=True
                )
                nc.vector.tensor_copy(out=bias_bc[:, ts(half, 512)], in_=psb)
            # K^T zero-padded per-head [128, H, T]; single slot (scores for
            # batch b finish before K chains of b+1 overwrite it).
            kT2 = wpool.tile([128, H, T], bf16, name="kT2")
            nc.gpsimd.memset(kT2, 0.0)
            # V stationary [s, j, h, 128]: cols 0:64 = 1.0, cols 64:128 = V.
            # Softmax row-sums land on psum partitions 0:64 (base 0 — the
            # custom-DVE reciprocal drops a nonzero input base partition),
            # raw output on 64:128. Two slots.
            v2_tiles = []
            for slot in range(2):
                t_ = wpool.tile([128, TCH, H, 128], bf16, name=f"v2_{slot}")
                nc.vector.memset(t_[:, :, :, 0:64], 1.0)
                v2_tiles.append(t_)

            qT_sb = wpool.tile([128, MO, T], bf16, name="qT_sb")
            outT_sb = wpool.tile([128, MO, T], bf16, name="outT_sb")

            def emit_scores_one(h, aT_map):
                pair = h // 2
                aT = apool.tile([128, 1280], bf16, name=f"aT{h}", tag=f"aT{h}")
                aT_map[h] = aT
                blk = psS.tile([128, 1280], f32, name="blk", tag="blk")
                for j in range(TCH):
                    nc.tensor.matmul(
                        blk[:, ds(off[j], widths[j])],
                        kT2[:, h, ts(j, 128)],
                        qT_sb[:, pair, ds(128 * j, widths[j])],
                        start=True,
                        stop=True,
                    )
                # one exp for the whole packed tile
                nc.scalar.activation(aT, blk, AF.Exp, scale=SCALE_QK)
                # zero the masked (s>t) part of the diagonal sub-blocks
                for j in range(TCH):
                    nc.gpsimd.tensor_mul(
                        aT[:, ds(off[j], 128)], aT[:, ds(off[j], 128)], mask_sb
                    )

            for b in range(b_loc):
                v2 = v2_tiles[b % 2]
                if b == 0:
                    xT_sb = xT0_sb
                    x8_sb = x80_sb
                else:
                    x8_sb = xpool.tile([128, GO, 2, T], f8e4, name="x8_sb", tag="x8")
                    for g in range(GO):
                        nc.sync.dma_start(
                            out=x8_sb[:, g, :, :], in_=x8[b, :, g, :, :]
                        )
                    xT_sb = xpool.tile([128, KO, T], bf16, name="xT_sb", tag="xT")
                    for k in range(KO):
                        nc.sync.dma_start(
                            out=xT_sb[:, k, :], in_=xT[b, ds(128 * k, 128), :]
                        )

                aT_map = {}
                # ---- QK phase with lagged scores+exp ----
                for m in range(MO):
                    ps = psB.tile([128, T], f32, name="ps_q", tag="ps1")
                    for g in range(GO):
                        nc.tensor.matmul(
                            ps,
                            wq_sb[:, g, :, ts(m, 128)],
                            x8_sb[:, g, :, :],
                            start=(g == 0),
                            stop=(g == GO - 1),
                            perf_mode=DR,
                        )
                    nc.vector.tensor_copy(out=qT_sb[:, m, :], in_=ps)
                    if m >= 1:
                        emit_scores_one(2 * (m - 1), aT_map)
                    ps = psB.tile([128, T], f32, name="ps_k", tag="ps1")
                    for g in range(GO):
                        nc.tensor.matmul(
                            ps,
                            wk_sb[:, g, :, ts(m, 128)],
                            x8_sb[:, g, :, :],
                            start=(g == 0),
                            stop=(g == GO - 1),
                            perf_mode=DR,
                        )
                    nc.vector.tensor_copy(out=kT2[0:64, 2 * m, :], in_=ps[0:64, :])
                    nc.vector.tensor_copy(
                        out=kT2[64:128, 2 * m + 1, :], in_=ps[64:128, :]
                    )
                    if m >= 1:
                        emit_scores_one(2 * m - 1, aT_map)

                # ---- V phase, half-major, with AV pairs interleaved ----
                def emit_v_chain(i, half):
                    ps = psB.tile([128, 512], f32, name="ps_v", tag="ps1")
                    for k in range(KO):
                        nc.tensor.matmul(
                            ps,
                            xT_sb[:, k, ts(i, 128)],
                            wv_sb[:, k, ts(half, 512)],
                            start=(k == 0),
                            stop=(k == KO - 1),
                        )
                    nc.scalar.copy(
                        out=v2[:, i, 8 * half : 8 * half + 8, 64:128],
                        in_=ps.rearrange("p (h d) -> p h d", d=64),
                    )

                def emit_av(h):
                    # AV transposed, windowed accumulation; psum rows 64:128
                    # carry the softmax row-sums (ones columns of v2).
                    pair, pb = h // 2, 64 * (h % 2)
                    av = psA.tile([128, T], f32, name="av", tag="av")
                    aT = aT_map[h]
                    for j in range(TCH):
                        nc.tensor.matmul(
                            av[:, ds(128 * j, widths[j])],
                            v2[:, j, h, :],
                            aT[:, ds(off[j], widths[j])],
                            start=(j == 0),
                            stop=(j == TCH - 1),
                        )
                    rec = spool.tile([64, T], f32, name="rec", tag="rec")
                    nc.vector.reciprocal_approx_fast(rec, av[0:64, :])
                    # normalization multiply split across DVE and GPSIMD.
                    # GPSIMD can't read PSUM, so ACT stages the back half
                    # into SBUF (bf16) first.
                    rawb = spool.tile([64, 256], bf16, name="rawb", tag="rawb")
                    nc.scalar.copy(out=rawb, in_=av[64:128, 256:512])
                    nc.vector.tensor_mul(
                        outT_sb[pb : pb + 64, pair, 0:256],
                        av[64:128, 0:256],
                        rec[:, 0:256],
                    )
                    nc.gpsimd.tensor_mul(
                        outT_sb[pb : pb + 64, pair, 256:512],
                        rawb,
                        rec[:, 256:512],
                    )

                emit_scores_one(14, aT_map)
                emit_v_chain(0, 0)
                emit_scores_one(15, aT_map)
                for i in range(1, TCH):
                    emit_v_chain(i, 0)
                for i in range(TCH):
                    emit_av(2 * i)
                    emit_av(2 * i + 1)
                    emit_v_chain(i, 1)
                for h in range(8, H):
                    emit_av(h)

                # ---- final projection; bias added during PSUM evacuation ----
                for i in range(TCH):
                    out_sb = opool.tile([128, C], f32, name="out_sb", tag="out_sb")
                    for half in range(2):
                        psF = psA.tile([128, 512], f32, name="psF", tag="av")
                        for k2 in range(MO):
                            nc.tensor.matmul(
                                psF,
                                outT_sb[:, k2, ts(i, 128)],
                                wp_sb[:, k2, ts(half, 512)],
                                start=(k2 == 0),
                                stop=(k2 == MO - 1),
                            )
                        nc.vector.tensor_add(
                            out=out_sb[:, ts(half, 512)],
                            in0=psF,
                            in1=bias_bc[:, ts(half, 512)],
                        )
                        nc.sync.dma_start(
                            out=out[b, ts(i, 128), ts(half, 512)],
                            in_=out_sb[:, ts(half, 512)],
                        )

    nc.compile()
    return nc


def make_in_maps(x, wq, wk, wv, w_proj, b_proj, b_loc=B_LOC, ncores=NCORES):
    bf16 = ml_dtypes.bfloat16
    f8 = ml_dtypes.float8_e4m3
    x = np.asarray(x, dtype=np.float32)
    # host-side layout prep (transpose / reshape / cast / scale only)
    xTf = np.ascontiguousarray(x.transpose(0, 2, 1))  # [B, C, T] f32
    xT = xTf.astype(bf16)
    # fp8 x for the DoubleRow Q/K projections: [B, ki, g, ko, T]
    x8 = np.ascontiguousarray(
        xTf.reshape(B, 4, 2, 128, T).transpose(0, 3, 1, 2, 4)
    ).astype(f8)
    # q,k weights x32 (fp8 subnormal avoidance), DoubleRow layout
    wq2f = np.asarray(wq, np.float32).transpose(1, 0, 2).reshape(C, HD) * 32.0
    wk2f = np.asarray(wk, np.float32).transpose(1, 0, 2).reshape(C, HD) * 32.0
    wq2 = np.ascontiguousarray(
        wq2f.reshape(4, 2, 128, HD).transpose(2, 0, 1, 3)
    ).astype(f8)
    wk2 = np.ascontiguousarray(
        wk2f.reshape(4, 2, 128, HD).transpose(2, 0, 1, 3)
    ).astype(f8)
    wv2 = np.ascontiguousarray(
        np.asarray(wv, np.float32).transpose(1, 0, 2).reshape(C, HD)
    ).astype(bf16)
    wp2 = np.ascontiguousarray(np.asarray(w_proj, np.float32)).astype(bf16)
    bp2 = np.asarray(b_proj, np.float32).reshape(1, C).astype(bf16)
    # mask[p, f] = 1 where p <= f (valid: s_in <= t_in on diagonal blocks)
    m = np.triu(np.ones((128, 128), np.float32)).astype(bf16)
    in_maps = []
    for c in range(ncores):
        in_maps.append(
            {
                "xT": xT[c * b_loc : (c + 1) * b_loc],
                "x8": x8[c * b_loc : (c + 1) * b_loc],
                "wq": wq2,
                "wk": wk2,
                "wv": wv2,
                "wp": wp2,
                "bp": bp2,
                "mask": m,
            }
        )
    return in_maps


def kernel(x, wq, wk, wv, w_proj, b_proj, **run_kwargs):
    from concourse import bass_utils

    if "nc" not in _CACHE:
        _CACHE["nc"] = build_nc(B_LOC)
    nc = _CACHE["nc"]
    in_maps = make_in_maps(x, wq, wk, wv, w_proj, b_proj)
    res = bass_utils.run_bass_kernel_spmd(
        nc, in_maps, core_ids=list(range(NCORES)), **run_kwargs
    )
    outs = [r["out"] for r in res.results]
    full = np.concatenate(outs, axis=0).astype(np.float32)
    if run_kwargs:
        _CACHE["last_result"] = res
    return full


# revision 3
# speedup vs baseline: 1.0225x; 1.0225x over previous
"""Causal multi-head attention (B=32,T=512,C=1024,H=16,D=64) on 8 TRN2 cores.

Data-parallel over batch (4 batches per core, no collectives). Per core:
  - Q/K projections run in fp8e4 DoubleRow (contraction pairs of 256),
    2.0x the bf16 matmul rate; weights are pre-scaled x32 on the host
    (fp8 subnormal avoidance) and the 1/(32*32) folds into the exp scale.
    V and the output projection stay bf16 for accuracy.
  - scores^T per head packed j0|j3|j1|j2 into one [128,1280] psum tile
    (each block inside a 2KB bank) -> a single Exp activation per head;
    causal diagonal blocks masked by GPSIMD multiplies.
  - scores+exp for head pair m-1 interleave into the QK phase (pattern
    [Q_m, sc_even, K_m, sc_odd]) so ACT's exp work hides under matmuls.
  - AV computed transposed (outT[d,t] = sum_s V[s,d] attnT[s,t]) with a
    [ones*64 | V] stationary: psum rows 0:64 get the softmax row-sums
    for free; rows 64:128 the raw output. Stream-dense (N=512..128
    windowed accumulation), produces the head-concat layout directly.
  - normalization: fast approx reciprocal (DVE) + multiply split across
    DVE (front half, PSUM-direct) and GPSIMD (back half via an ACT
    PSUM->SBUF stage); AV pairs interleave into the V-projection phase.
  - final projection overlaps the AV tail (i=0 chain's first 6 chunks
    only need heads 0..11); bias added during PSUM evacuation; bf16
    output DMA, upcast to f32 on the host.
  - DMA order: fp8 x first, then wq/wk interleaved per m-chunk
    (m-major layout) so projection chain m never waits on later chunks.
"""

import sys

if "/opt/trn_rl_repo" not in sys.path:
    sys.path.insert(0, "/opt/trn_rl_repo")

import numpy as np
import ml_dtypes

B, T, C = 32, 512, 1024
H, D = 16, 64
HD = H * D
MO = HD // 128
NCORES = 8
B_LOC = B // NCORES

_CACHE = {}


def build_nc(b_loc=B_LOC):
    import concourse.mybir as mybir
    from concourse import bacc
    from concourse.bass import ds, ts
    from concourse.tile import TileContext

    f32 = mybir.dt.float32
    bf16 = mybir.dt.bfloat16
    f8e4 = mybir.dt.float8e4
    DR = mybir.MatmulPerfMode.DoubleRow
    AF = mybir.ActivationFunctionType

    KO = C // 128  # 8 contraction chunks
    GO = C // 256  # 4 double-row contraction groups
    MO = HD // 128  # 8 output-row chunks
    TCH = T // 128  # 4 t-chunks
    SCALE = 1.0 / float(np.sqrt(C))
    SCALE_QK = SCALE / 1024.0

    widths = [T - 128 * j for j in range(TCH)]  # 512, 384, 256, 128
    # scores packed j0|j3|j1|j2 in ONE [128,1280] psum tile: every block's
    # matmul dst stays inside a 2KB bank (j0@[0,512) b0, j3@[512,640) b1,
    # j1@[640,1024) b1, j2@[1024,1280) b2) -> one exp per head.
    off = [0, 640, 1024, 512]

    nc = bacc.Bacc("TRN2", target_bir_lowering=False)
    xT = nc.dram_tensor("xT", [b_loc, C, T], bf16, kind="ExternalInput")
    x8 = nc.dram_tensor("x8", [b_loc, 128, GO, 2, T], f8e4, kind="ExternalInput")
    wq = nc.dram_tensor("wq", [MO, 128, GO, 2, 128], f8e4, kind="ExternalInput")
    wk = nc.dram_tensor("wk", [MO, 128, GO, 2, 128], f8e4, kind="ExternalInput")
    wv = nc.dram_tensor("wv", [C, HD], bf16, kind="ExternalInput")
    wp = nc.dram_tensor("wp", [C, C], bf16, kind="ExternalInput")
    bp = nc.dram_tensor("bp", [1, C], bf16, kind="ExternalInput")
    mask = nc.dram_tensor("mask", [128, 128], bf16, kind="ExternalInput")
    out = nc.dram_tensor("out", [b_loc, T, C], bf16, kind="ExternalOutput")

    with TileContext(nc) as tc:
        with (
            tc.tile_pool(name="weights", bufs=1) as wpool,
            tc.tile_pool(name="acts", bufs=2) as xpool,
            tc.tile_pool(name="attn", bufs=1) as apool,
            tc.tile_pool(name="small", bufs=3) as spool,
            tc.tile_pool(name="outs", bufs=2) as opool,
            tc.tile_pool(name="psS", bufs=1, space="PSUM") as psS,
            tc.tile_pool(name="psAV", bufs=3, space="PSUM") as psA,
            tc.tile_pool(name="ps1", bufs=2, space="PSUM") as psB,
        ):
            # ---- persistent weights / constants ----
            wq_sb = wpool.tile([128, MO, GO, 2, 128], f8e4, name="wq_sb")
            wk_sb = wpool.tile([128, MO, GO, 2, 128], f8e4, name="wk_sb")
            wv_sb = wpool.tile([128, KO, HD], bf16, name="wv_sb")
            wp_sb = wpool.tile([128, KO, C], bf16, name="wp_sb")
            xT0_sb = xpool.tile([128, KO, T], bf16, name="xT0_sb", tag="xT")
            x80_sb = xpool.tile([128, GO, 2, T], f8e4, name="x80_sb", tag="x8")
            bp1_sb = wpool.tile([1, C], bf16, name="bp1_sb")
            nc.sync.dma_start(out=bp1_sb, in_=bp[:])
            mask_sb = wpool.tile([128, 128], bf16, name="mask_sb")
            nc.sync.dma_start(out=mask_sb, in_=mask[:])
            # fp8 x first, then wq/wk interleaved per m-chunk so chain m
            # never waits on later chunks
            for g in range(GO):
                nc.sync.dma_start(out=x80_sb[:, g, :, :], in_=x8[0, :, g, :, :])
            for m in range(MO):
                nc.sync.dma_start(out=wq_sb[:, m, :, :, :], in_=wq[m])
                nc.sync.dma_start(out=wk_sb[:, m, :, :, :], in_=wk[m])
            for k in range(KO):
                nc.sync.dma_start(out=xT0_sb[:, k, :], in_=xT[0, ds(128 * k, 128), :])
            for k in range(KO):
                nc.sync.dma_start(out=wv_sb[:, k, :], in_=wv[ds(128 * k, 128), :])
            nc.sync.dma_start(
                out=wp_sb, in_=wp[:].rearrange("(ko p) n -> p ko n", p=128)
            )
            # bias broadcast to all 128 partitions, f32, built once
            ones1_sb = wpool.tile([1, 128], bf16, name="ones1_sb")
            nc.gpsimd.memset(ones1_sb, 1.0)
            bias_bc = wpool.tile([128, C], f32, name="bias_bc")
            for half in range(2):
                psb = psB.tile([128, 512], f32, name="psb", tag="ps1")
                nc.tensor.matmul(
                    psb, ones1_sb, bp1_sb[:, ts(half, 512)], start=True, stop=True
                )
                nc.vector.tensor_copy(out=bias_bc[:, ts(half, 512)], in_=psb)
            # K^T zero-padded per-head [128, H, T]; single slot (scores for
            # batch b finish before K chains of b+1 overwrite it).
            kT2 = wpool.tile([128, H, T], bf16, name="kT2")
            nc.gpsimd.memset(kT2, 0.0)
            # V stationary [s, j, h, 128]: cols 0:64 = 1.0, cols 64:128 = V.
            # Softmax row-sums land on psum partitions 0:64 (base 0 — the
            # custom-DVE reciprocal drops a nonzero input base partition),
            # raw output on 64:128. Two slots.
            v2_tiles = []
            for slot in range(2):
                t_ = wpool.tile([128, TCH, H, 128], bf16, name=f"v2_{slot}")
                nc.vector.memset(t_[:, :, :, 0:64], 1.0)
                v2_tiles.append(t_)

            qT_sb = wpool.tile([128, MO, T], bf16, name="qT_sb")
            outT_sb = wpool.tile([128, MO, T], bf16, name="outT_sb")

            def emit_scores_one(h, aT_map):
                pair = h // 2
                aT = apool.tile([128, 1280], bf16, name=f"aT{h}", tag=f"aT{h}")
                aT_map[h] = aT
                blk = psS.tile([128, 1280], f32, name="blk", tag="blk")
                for j in range(TCH):
                    nc.tensor.matmul(
                        blk[:, ds(off[j], widths[j])],
                        kT2[:, h, ts(j, 128)],
                        qT_sb[:, pair, ds(128 * j, widths[j])],
                        start=True,
                        stop=True,
                    )
                # one exp for the whole packed tile
                nc.scalar.activation(aT, blk, AF.Exp, scale=SCALE_QK)
                # zero the masked (s>t) part of the diagonal sub-blocks
                for j in range(TCH):
                    nc.gpsimd.tensor_mul(
                        aT[:, ds(off[j], 128)], aT[:, ds(off[j], 128)], mask_sb
                    )

            for b in range(b_loc):
                v2 = v2_tiles[b % 2]
                if b == 0:
                    xT_sb = xT0_sb
                    x8_sb = x80_sb
                else:
                    x8_sb = xpool.tile([128, GO, 2, T], f8e4, name="x8_sb", tag="x8")
                    for g in range(GO):
                        nc.sync.dma_start(
                            out=x8_sb[:, g, :, :], in_=x8[b, :, g, :, :]
                        )
                    xT_sb = xpool.tile([128, KO, T], bf16, name="xT_sb", tag="xT")
                    for k in range(KO):
                        nc.sync.dma_start(
                            out=xT_sb[:, k, :], in_=xT[b, ds(128 * k, 128), :]
                        )

                aT_map = {}
                # ---- QK phase with lagged scores+exp ----
                for m in range(MO):
                    ps = psB.tile([128, T], f32, name="ps_q", tag="ps1")
                    for g in range(GO):
                        nc.tensor.matmul(
                            ps,
                            wq_sb[:, m, g, :, :],
                            x8_sb[:, g, :, :],
                            start=(g == 0),
                            stop=(g == GO - 1),
                            perf_mode=DR,
                        )
                    nc.vector.tensor_copy(out=qT_sb[:, m, :], in_=ps)
                    if m >= 1:
                        emit_scores_one(2 * (m - 1), aT_map)
                    ps = psB.tile([128, T], f32, name="ps_k", tag="ps1")
                    for g in range(GO):
                        nc.tensor.matmul(
                            ps,
                            wk_sb[:, m, g, :, :],
                            x8_sb[:, g, :, :],
                            start=(g == 0),
                            stop=(g == GO - 1),
                            perf_mode=DR,
                        )
                    nc.vector.tensor_copy(out=kT2[0:64, 2 * m, :], in_=ps[0:64, :])
                    nc.vector.tensor_copy(
                        out=kT2[64:128, 2 * m + 1, :], in_=ps[64:128, :]
                    )
                    if m >= 1:
                        emit_scores_one(2 * m - 1, aT_map)

                # ---- V phase, half-major, with AV pairs interleaved ----
                def emit_v_chain(i, half):
                    ps = psB.tile([128, 512], f32, name="ps_v", tag="ps1")
                    for k in range(KO):
                        nc.tensor.matmul(
                            ps,
                            xT_sb[:, k, ts(i, 128)],
                            wv_sb[:, k, ts(half, 512)],
                            start=(k == 0),
                            stop=(k == KO - 1),
                        )
                    nc.scalar.copy(
                        out=v2[:, i, 8 * half : 8 * half + 8, 64:128],
                        in_=ps.rearrange("p (h d) -> p h d", d=64),
                    )

                def emit_av(h):
                    # AV transposed, windowed accumulation; psum rows 64:128
                    # carry the softmax row-sums (ones columns of v2).
                    pair, pb = h // 2, 64 * (h % 2)
                    av = psA.tile([128, T], f32, name="av", tag="av")
                    aT = aT_map[h]
                    for j in range(TCH):
                        nc.tensor.matmul(
                            av[:, ds(128 * j, widths[j])],
                            v2[:, j, h, :],
                            aT[:, ds(off[j], widths[j])],
                            start=(j == 0),
                            stop=(j == TCH - 1),
                        )
                    rec = spool.tile([64, T], f32, name="rec", tag="rec")
                    nc.vector.reciprocal_approx_fast(rec, av[0:64, :])
                    # normalization multiply split across DVE and GPSIMD.
                    # GPSIMD can't read PSUM, so ACT stages the back half
                    # into SBUF (bf16) first.
                    rawb = spool.tile([64, 256], bf16, name="rawb", tag="rawb")
                    nc.scalar.copy(out=rawb, in_=av[64:128, 256:512])
                    nc.vector.tensor_mul(
                        outT_sb[pb : pb + 64, pair, 0:256],
                        av[64:128, 0:256],
                        rec[:, 0:256],
                    )
                    nc.gpsimd.tensor_mul(
                        outT_sb[pb : pb + 64, pair, 256:512],
                        rawb,
                        rec[:, 256:512],
                    )

                emit_scores_one(14, aT_map)
                emit_v_chain(0, 0)
                emit_scores_one(15, aT_map)
                for i in range(1, TCH):
                    emit_v_chain(i, 0)
                for i in range(TCH):
                    emit_av(2 * i)
                    emit_av(2 * i + 1)
                    emit_v_chain(i, 1)
                for h in range(8, 12):
                    emit_av(h)
                # start the i=0 projection early: chunks k2=0..5 only need
                # heads 0..11; runs on the (now idle) QKV psum slots
                psF0 = []
                for half in range(2):
                    psF = psB.tile([128, 512], f32, name="psF0", tag="ps1")
                    psF0.append(psF)
                    for k2 in range(6):
                        nc.tensor.matmul(
                            psF,
                            outT_sb[:, k2, ts(0, 128)],
                            wp_sb[:, k2, ts(half, 512)],
                            start=(k2 == 0),
                            stop=False,
                            skip_group_check=True,
                        )
                for h in range(12, H):
                    emit_av(h)
                out_sb0 = opool.tile([128, C], bf16, name="out_sb0", tag="out_sb")
                for half in range(2):
                    for k2 in range(6, MO):
                        nc.tensor.matmul(
                            psF0[half],
                            outT_sb[:, k2, ts(0, 128)],
                            wp_sb[:, k2, ts(half, 512)],
                            start=False,
                            stop=(k2 == MO - 1),
                            skip_group_check=True,
                        )
                    nc.vector.tensor_add(
                        out=out_sb0[:, ts(half, 512)],
                        in0=psF0[half],
                        in1=bias_bc[:, ts(half, 512)],
                    )
                    nc.sync.dma_start(
                        out=out[b, ts(0, 128), ts(half, 512)],
                        in_=out_sb0[:, ts(half, 512)],
                    )

                # ---- final projection; bias added during PSUM evacuation ----
                for i in range(1, TCH):
                    out_sb = opool.tile([128, C], bf16, name="out_sb", tag="out_sb")
                    for half in range(2):
                        psF = psA.tile([128, 512], f32, name="psF", tag="av")
                        for k2 in range(MO):
                            nc.tensor.matmul(
                                psF,
                                outT_sb[:, k2, ts(i, 128)],
                                wp_sb[:, k2, ts(half, 512)],
                                start=(k2 == 0),
                                stop=(k2 == MO - 1),
                            )
                        nc.vector.tensor_add(
                            out=out_sb[:, ts(half, 512)],
                            in0=psF,
                            in1=bias_bc[:, ts(half, 512)],
                        )
                        nc.sync.dma_start(
                            out=out[b, ts(i, 128), ts(half, 512)],
                            in_=out_sb[:, ts(half, 512)],
                        )

    nc.compile()
    return nc


def make_in_maps(x, wq, wk, wv, w_proj, b_proj, b_loc=B_LOC, ncores=NCORES):
    bf16 = ml_dtypes.bfloat16
    f8 = ml_dtypes.float8_e4m3
    x = np.asarray(x, dtype=np.float32)
    # host-side layout prep (transpose / reshape / cast / scale only)
    xTf = np.ascontiguousarray(x.transpose(0, 2, 1))  # [B, C, T] f32
    xT = xTf.astype(bf16)
    # fp8 x for the DoubleRow Q/K projections: [B, ki, g, ko, T]
    x8 = np.ascontiguousarray(
        xTf.reshape(B, 4, 2, 128, T).transpose(0, 3, 1, 2, 4)
    ).astype(f8)
    # q,k weights x32 (fp8 subnormal avoidance), DoubleRow layout
    wq2f = np.asarray(wq, np.float32).transpose(1, 0, 2).reshape(C, HD) * 32.0
    wk2f = np.asarray(wk, np.float32).transpose(1, 0, 2).reshape(C, HD) * 32.0
    # [g, ko, ki, m*128] -> [m, ki, g, ko, 128]
    wq2 = np.ascontiguousarray(
        wq2f.reshape(4, 2, 128, MO, 128).transpose(3, 2, 0, 1, 4)
    ).astype(f8)
    wk2 = np.ascontiguousarray(
        wk2f.reshape(4, 2, 128, MO, 128).transpose(3, 2, 0, 1, 4)
    ).astype(f8)
    wv2 = np.ascontiguousarray(
        np.asarray(wv, np.float32).transpose(1, 0, 2).reshape(C, HD)
    ).astype(bf16)
    wp2 = np.ascontiguousarray(np.asarray(w_proj, np.float32)).astype(bf16)
    bp2 = np.asarray(b_proj, np.float32).reshape(1, C).astype(bf16)
    # mask[p, f] = 1 where p <= f (valid: s_in <= t_in on diagonal blocks)
    m = np.triu(np.ones((128, 128), np.float32)).astype(bf16)
    in_maps = []
    for c in range(ncores):
        in_maps.append(
            {
                "xT": xT[c * b_loc : (c + 1) * b_loc],
                "x8": x8[c * b_loc : (c + 1) * b_loc],
                "wq": wq2,
                "wk": wk2,
                "wv": wv2,
                "wp": wp2,
                "bp": bp2,
                "mask": m,
            }
        )
    return in_maps


def kernel(x, wq, wk, wv, w_proj, b_proj, **run_kwargs):
    from concourse import bass_utils

    if "nc" not in _CACHE:
        _CACHE["nc"] = build_nc(B_LOC)
    nc = _CACHE["nc"]
    in_maps = make_in_maps(x, wq, wk, wv, w_proj, b_proj)
    res = bass_utils.run_bass_kernel_spmd(
        nc, in_maps, core_ids=list(range(NCORES)), **run_kwargs
    )
    outs = [np.asarray(r["out"], dtype=np.float32) for r in res.results]
    full = np.concatenate(outs, axis=0).astype(np.float32)
    if run_kwargs:
        _CACHE["last_result"] = res
    return full


# revision 4
# speedup vs baseline: 1.0234x; 1.0008x over previous
"""Causal multi-head attention (B=32,T=512,C=1024,H=16,D=64) on 8 TRN2 cores.

Data-parallel over batch (4 batches per core, no collectives). Per core:
  - Q/K projections run in fp8e4 DoubleRow (contraction pairs of 256),
    2.0x the bf16 matmul rate; weights are pre-scaled x32 on the host
    (fp8 subnormal avoidance) and the 1/(32*32) folds into the exp scale.
    V and the output projection stay bf16 for accuracy.
  - scores^T per head packed j0|j3|j1|j2 into one [128,1280] psum tile
    (each block inside a 2KB bank) -> a single Exp activation per head;
    causal diagonal blocks masked by GPSIMD multiplies.
  - scores+exp for head pair m-1 interleave into the QK phase (pattern
    [Q_m, sc_even, K_m, sc_odd]) so ACT's exp work hides under matmuls.
  - AV computed transposed (outT[d,t] = sum_s V[s,d] attnT[s,t]) with a
    [ones*64 | V] stationary: psum rows 0:64 get the softmax row-sums
    for free; rows 64:128 the raw output. Stream-dense (N=512..128
    windowed accumulation), produces the head-concat layout directly.
  - normalization: fast approx reciprocal (DVE) + multiply split across
    DVE (front half, PSUM-direct) and GPSIMD (back half via an ACT
    PSUM->SBUF stage); AV pairs interleave into the V-projection phase.
  - final projection overlaps the AV tail (i=0 chain's first 6 chunks
    only need heads 0..11); bias added during PSUM evacuation; bf16
    output DMA, upcast to f32 on the host.
  - DMA order: fp8 x first, then wq/wk interleaved per m-chunk
    (m-major layout) so projection chain m never waits on later chunks;
    batch b+1's x prefetches ahead of batch b's output DMAs in the
    in-order sync queue.
"""

import sys

if "/opt/trn_rl_repo" not in sys.path:
    sys.path.insert(0, "/opt/trn_rl_repo")

import numpy as np
import ml_dtypes

B, T, C = 32, 512, 1024
H, D = 16, 64
HD = H * D
MO = HD // 128
NCORES = 8
B_LOC = B // NCORES

_CACHE = {}


def build_nc(b_loc=B_LOC):
    import concourse.mybir as mybir
    from concourse import bacc
    from concourse.bass import ds, ts
    from concourse.tile import TileContext

    f32 = mybir.dt.float32
    bf16 = mybir.dt.bfloat16
    f8e4 = mybir.dt.float8e4
    DR = mybir.MatmulPerfMode.DoubleRow
    AF = mybir.ActivationFunctionType

    KO = C // 128  # 8 contraction chunks
    GO = C // 256  # 4 double-row contraction groups
    MO = HD // 128  # 8 output-row chunks
    TCH = T // 128  # 4 t-chunks
    SCALE = 1.0 / float(np.sqrt(C))
    SCALE_QK = SCALE / 1024.0

    widths = [T - 128 * j for j in range(TCH)]  # 512, 384, 256, 128
    # scores packed j0|j3|j1|j2 in ONE [128,1280] psum tile: every block's
    # matmul dst stays inside a 2KB bank (j0@[0,512) b0, j3@[512,640) b1,
    # j1@[640,1024) b1, j2@[1024,1280) b2) -> one exp per head.
    off = [0, 640, 1024, 512]

    nc = bacc.Bacc("TRN2", target_bir_lowering=False)
    xT = nc.dram_tensor("xT", [b_loc, C, T], bf16, kind="ExternalInput")
    x8 = nc.dram_tensor("x8", [b_loc, 128, GO, 2, T], f8e4, kind="ExternalInput")
    wq = nc.dram_tensor("wq", [MO, 128, GO, 2, 128], f8e4, kind="ExternalInput")
    wk = nc.dram_tensor("wk", [MO, 128, GO, 2, 128], f8e4, kind="ExternalInput")
    wv = nc.dram_tensor("wv", [C, HD], bf16, kind="ExternalInput")
    wp = nc.dram_tensor("wp", [C, C], bf16, kind="ExternalInput")
    bp = nc.dram_tensor("bp", [1, C], bf16, kind="ExternalInput")
    mask = nc.dram_tensor("mask", [128, 128], bf16, kind="ExternalInput")
    out = nc.dram_tensor("out", [b_loc, T, C], bf16, kind="ExternalOutput")

    with TileContext(nc) as tc:
        with (
            tc.tile_pool(name="weights", bufs=1) as wpool,
            tc.tile_pool(name="acts", bufs=2) as xpool,
            tc.tile_pool(name="attn", bufs=1) as apool,
            tc.tile_pool(name="small", bufs=3) as spool,
            tc.tile_pool(name="outs", bufs=2) as opool,
            tc.tile_pool(name="psS", bufs=1, space="PSUM") as psS,
            tc.tile_pool(name="psAV", bufs=3, space="PSUM") as psA,
            tc.tile_pool(name="ps1", bufs=2, space="PSUM") as psB,
        ):
            # ---- persistent weights / constants ----
            wq_sb = wpool.tile([128, MO, GO, 2, 128], f8e4, name="wq_sb")
            wk_sb = wpool.tile([128, MO, GO, 2, 128], f8e4, name="wk_sb")
            wv_sb = wpool.tile([128, KO, HD], bf16, name="wv_sb")
            wp_sb = wpool.tile([128, KO, C], bf16, name="wp_sb")
            xT0_sb = xpool.tile([128, KO, T], bf16, name="xT0_sb", tag="xT")
            x80_sb = xpool.tile([128, GO, 2, T], f8e4, name="x80_sb", tag="x8")
            bp1_sb = wpool.tile([1, C], bf16, name="bp1_sb")
            nc.sync.dma_start(out=bp1_sb, in_=bp[:])
            mask_sb = wpool.tile([128, 128], bf16, name="mask_sb")
            nc.sync.dma_start(out=mask_sb, in_=mask[:])
            # fp8 x first, then wq/wk interleaved per m-chunk so chain m
            # never waits on later chunks
            for g in range(GO):
                nc.sync.dma_start(out=x80_sb[:, g, :, :], in_=x8[0, :, g, :, :])
            for m in range(MO):
                nc.sync.dma_start(out=wq_sb[:, m, :, :, :], in_=wq[m])
                nc.sync.dma_start(out=wk_sb[:, m, :, :, :], in_=wk[m])
            for k in range(KO):
                nc.sync.dma_start(out=xT0_sb[:, k, :], in_=xT[0, ds(128 * k, 128), :])
            for k in range(KO):
                nc.sync.dma_start(out=wv_sb[:, k, :], in_=wv[ds(128 * k, 128), :])
            nc.sync.dma_start(
                out=wp_sb, in_=wp[:].rearrange("(ko p) n -> p ko n", p=128)
            )
            # bias broadcast to all 128 partitions, f32, built once
            ones1_sb = wpool.tile([1, 128], bf16, name="ones1_sb")
            nc.gpsimd.memset(ones1_sb, 1.0)
            bias_bc = wpool.tile([128, C], f32, name="bias_bc")
            for half in range(2):
                psb = psB.tile([128, 512], f32, name="psb", tag="ps1")
                nc.tensor.matmul(
                    psb, ones1_sb, bp1_sb[:, ts(half, 512)], start=True, stop=True
                )
                nc.vector.tensor_copy(out=bias_bc[:, ts(half, 512)], in_=psb)
            # K^T zero-padded per-head [128, H, T]; single slot (scores for
            # batch b finish before K chains of b+1 overwrite it).
            kT2 = wpool.tile([128, H, T], bf16, name="kT2")
            nc.gpsimd.memset(kT2, 0.0)
            # V stationary [s, j, h, 128]: cols 0:64 = 1.0, cols 64:128 = V.
            # Softmax row-sums land on psum partitions 0:64 (base 0 — the
            # custom-DVE reciprocal drops a nonzero input base partition),
            # raw output on 64:128. Two slots.
            v2_tiles = []
            for slot in range(2):
                t_ = wpool.tile([128, TCH, H, 128], bf16, name=f"v2_{slot}")
                nc.vector.memset(t_[:, :, :, 0:64], 1.0)
                v2_tiles.append(t_)

            qT_sb = wpool.tile([128, MO, T], bf16, name="qT_sb")
            outT_sb = wpool.tile([128, MO, T], bf16, name="outT_sb")

            def emit_scores_one(h, aT_map):
                pair = h // 2
                aT = apool.tile([128, 1280], bf16, name=f"aT{h}", tag=f"aT{h}")
                aT_map[h] = aT
                blk = psS.tile([128, 1280], f32, name="blk", tag="blk")
                for j in range(TCH):
                    nc.tensor.matmul(
                        blk[:, ds(off[j], widths[j])],
                        kT2[:, h, ts(j, 128)],
                        qT_sb[:, pair, ds(128 * j, widths[j])],
                        start=True,
                        stop=True,
                    )
                # one exp for the whole packed tile
                nc.scalar.activation(aT, blk, AF.Exp, scale=SCALE_QK)
                # zero the masked (s>t) part of the diagonal sub-blocks
                for j in range(TCH):
                    nc.gpsimd.tensor_mul(
                        aT[:, ds(off[j], 128)], aT[:, ds(off[j], 128)], mask_sb
                    )

            x_tiles = {0: (xT0_sb, x80_sb)}
            for b in range(b_loc):
                v2 = v2_tiles[b % 2]
                xT_sb, x8_sb = x_tiles[b]
                if b + 1 < b_loc:
                    # prefetch next batch's x ahead of this batch's output
                    # DMAs in the in-order sync queue
                    x8n = xpool.tile([128, GO, 2, T], f8e4, name="x8_sb", tag="x8")
                    for g in range(GO):
                        nc.sync.dma_start(
                            out=x8n[:, g, :, :], in_=x8[b + 1, :, g, :, :]
                        )
                    xTn = xpool.tile([128, KO, T], bf16, name="xT_sb", tag="xT")
                    for k in range(KO):
                        nc.sync.dma_start(
                            out=xTn[:, k, :], in_=xT[b + 1, ds(128 * k, 128), :]
                        )
                    x_tiles[b + 1] = (xTn, x8n)

                aT_map = {}
                # ---- QK phase with lagged scores+exp ----
                for m in range(MO):
                    ps = psB.tile([128, T], f32, name="ps_q", tag="ps1")
                    for g in range(GO):
                        nc.tensor.matmul(
                            ps,
                            wq_sb[:, m, g, :, :],
                            x8_sb[:, g, :, :],
                            start=(g == 0),
                            stop=(g == GO - 1),
                            perf_mode=DR,
                        )
                    nc.vector.tensor_copy(out=qT_sb[:, m, :], in_=ps)
                    if m >= 1:
                        emit_scores_one(2 * (m - 1), aT_map)
                    ps = psB.tile([128, T], f32, name="ps_k", tag="ps1")
                    for g in range(GO):
                        nc.tensor.matmul(
                            ps,
                            wk_sb[:, m, g, :, :],
                            x8_sb[:, g, :, :],
                            start=(g == 0),
                            stop=(g == GO - 1),
                            perf_mode=DR,
                        )
                    nc.vector.tensor_copy(out=kT2[0:64, 2 * m, :], in_=ps[0:64, :])
                    nc.vector.tensor_copy(
                        out=kT2[64:128, 2 * m + 1, :], in_=ps[64:128, :]
                    )
                    if m >= 1:
                        emit_scores_one(2 * m - 1, aT_map)

                # ---- V phase, half-major, with AV pairs interleaved ----
                def emit_v_chain(i, half):
                    ps = psB.tile([128, 512], f32, name="ps_v", tag="ps1")
                    for k in range(KO):
                        nc.tensor.matmul(
                            ps,
                            xT_sb[:, k, ts(i, 128)],
                            wv_sb[:, k, ts(half, 512)],
                            start=(k == 0),
                            stop=(k == KO - 1),
                        )
                    nc.scalar.copy(
                        out=v2[:, i, 8 * half : 8 * half + 8, 64:128],
                        in_=ps.rearrange("p (h d) -> p h d", d=64),
                    )

                def emit_av(h):
                    # AV transposed, windowed accumulation; psum rows 64:128
                    # carry the softmax row-sums (ones columns of v2).
                    pair, pb = h // 2, 64 * (h % 2)
                    av = psA.tile([128, T], f32, name="av", tag="av")
                    aT = aT_map[h]
                    for j in range(TCH):
                        nc.tensor.matmul(
                            av[:, ds(128 * j, widths[j])],
                            v2[:, j, h, :],
                            aT[:, ds(off[j], widths[j])],
                            start=(j == 0),
                            stop=(j == TCH - 1),
                        )
                    rec = spool.tile([64, T], f32, name="rec", tag="rec")
                    nc.vector.reciprocal_approx_fast(rec, av[0:64, :])
                    # normalization multiply split across DVE and GPSIMD.
                    # GPSIMD can't read PSUM, so ACT stages the back half
                    # into SBUF (bf16) first.
                    rawb = spool.tile([64, 256], bf16, name="rawb", tag="rawb")
                    nc.scalar.copy(out=rawb, in_=av[64:128, 256:512])
                    nc.vector.tensor_mul(
                        outT_sb[pb : pb + 64, pair, 0:256],
                        av[64:128, 0:256],
                        rec[:, 0:256],
                    )
                    nc.gpsimd.tensor_mul(
                        outT_sb[pb : pb + 64, pair, 256:512],
                        rawb,
                        rec[:, 256:512],
                    )

                emit_scores_one(14, aT_map)
                emit_v_chain(0, 0)
                emit_scores_one(15, aT_map)
                for i in range(1, TCH):
                    emit_v_chain(i, 0)
                for i in range(TCH):
                    emit_av(2 * i)
                    emit_av(2 * i + 1)
                    emit_v_chain(i, 1)
                for h in range(8, 12):
                    emit_av(h)
                # start the i=0 projection early: chunks k2=0..5 only need
                # heads 0..11; runs on the (now idle) QKV psum slots
                psF0 = []
                for half in range(2):
                    psF = psB.tile([128, 512], f32, name="psF0", tag="ps1")
                    psF0.append(psF)
                    for k2 in range(6):
                        nc.tensor.matmul(
                            psF,
                            outT_sb[:, k2, ts(0, 128)],
                            wp_sb[:, k2, ts(half, 512)],
                            start=(k2 == 0),
                            stop=False,
                            skip_group_check=True,
                        )
                for h in range(12, H):
                    emit_av(h)
                out_sb0 = opool.tile([128, C], bf16, name="out_sb0", tag="out_sb")
                for half in range(2):
                    for k2 in range(6, MO):
                        nc.tensor.matmul(
                            psF0[half],
                            outT_sb[:, k2, ts(0, 128)],
                            wp_sb[:, k2, ts(half, 512)],
                            start=False,
                            stop=(k2 == MO - 1),
                            skip_group_check=True,
                        )
                    nc.vector.tensor_add(
                        out=out_sb0[:, ts(half, 512)],
                        in0=psF0[half],
                        in1=bias_bc[:, ts(half, 512)],
                    )
                    nc.sync.dma_start(
                        out=out[b, ts(0, 128), ts(half, 512)],
                        in_=out_sb0[:, ts(half, 512)],
                    )

                # ---- final projection; bias added during PSUM evacuation ----
                for i in range(1, TCH):
                    out_sb = opool.tile([128, C], bf16, name="out_sb", tag="out_sb")
                    for half in range(2):
                        psF = psA.tile([128, 512], f32, name="psF", tag="av")
                        for k2 in range(MO):
                            nc.tensor.matmul(
                                psF,
                                outT_sb[:, k2, ts(i, 128)],
                                wp_sb[:, k2, ts(half, 512)],
                                start=(k2 == 0),
                                stop=(k2 == MO - 1),
                            )
                        nc.vector.tensor_add(
                            out=out_sb[:, ts(half, 512)],
                            in0=psF,
                            in1=bias_bc[:, ts(half, 512)],
                        )
                        nc.sync.dma_start(
                            out=out[b, ts(i, 128), ts(half, 512)],
                            in_=out_sb[:, ts(half, 512)],
                        )

    nc.compile()
    return nc


def make_in_maps(x, wq, wk, wv, w_proj, b_proj, b_loc=B_LOC, ncores=NCORES):
    bf16 = ml_dtypes.bfloat16
    f8 = ml_dtypes.float8_e4m3
    x = np.asarray(x, dtype=np.float32)
    # host-side layout prep (transpose / reshape / cast / scale only)
    xTf = np.ascontiguousarray(x.transpose(0, 2, 1))  # [B, C, T] f32
    xT = xTf.astype(bf16)
    # fp8 x for the DoubleRow Q/K projections: [B, ki, g, ko, T]
    x8 = np.ascontiguousarray(
        xTf.reshape(B, 4, 2, 128, T).transpose(0, 3, 1, 2, 4)
    ).astype(f8)
    # q,k weights x32 (fp8 subnormal avoidance), DoubleRow layout
    wq2f = np.asarray(wq, np.float32).transpose(1, 0, 2).reshape(C, HD) * 32.0
    wk2f = np.asarray(wk, np.float32).transpose(1, 0, 2).reshape(C, HD) * 32.0
    # [g, ko, ki, m*128] -> [m, ki, g, ko, 128]
    wq2 = np.ascontiguousarray(
        wq2f.reshape(4, 2, 128, MO, 128).transpose(3, 2, 0, 1, 4)
    ).astype(f8)
    wk2 = np.ascontiguousarray(
        wk2f.reshape(4, 2, 128, MO, 128).transpose(3, 2, 0, 1, 4)
    ).astype(f8)
    wv2 = np.ascontiguousarray(
        np.asarray(wv, np.float32).transpose(1, 0, 2).reshape(C, HD)
    ).astype(bf16)
    wp2 = np.ascontiguousarray(np.asarray(w_proj, np.float32)).astype(bf16)
    bp2 = np.asarray(b_proj, np.float32).reshape(1, C).astype(bf16)
    # mask[p, f] = 1 where p <= f (valid: s_in <= t_in on diagonal blocks)
    m = np.triu(np.ones((128, 128), np.float32)).astype(bf16)
    in_maps = []
    for c in range(ncores):
        in_maps.append(
            {
                "xT": xT[c * b_loc : (c + 1) * b_loc],
                "x8": x8[c * b_loc : (c + 1) * b_loc],
                "wq": wq2,
                "wk": wk2,
                "wv": wv2,
                "wp": wp2,
                "bp": bp2,
                "mask": m,
            }
        )
    return in_maps


def kernel(x, wq, wk, wv, w_proj, b_proj, **run_kwargs):
    from concourse import bass_utils

    if "nc" not in _CACHE:
        _CACHE["nc"] = build_nc(B_LOC)
    nc = _CACHE["nc"]
    in_maps = make_in_maps(x, wq, wk, wv, w_proj, b_proj)
    res = bass_utils.run_bass_kernel_spmd(
        nc, in_maps, core_ids=list(range(NCORES)), **run_kwargs
    )
    outs = [np.asarray(r["out"], dtype=np.float32) for r in res.results]
    full = np.concatenate(outs, axis=0).astype(np.float32)
    if run_kwargs:
        _CACHE["last_result"] = res
    return full
